# revision 1
# baseline (speedup 1.0000x reference)
"""Trainium2 Bass kernel for time-decayed causal KNN retrieval + fusion scoring.

Math (reference):
  sim_t[i,j] = cos(q_i, p_j) * exp(-l*|ti-tj|)
  masked     = causal(tj < ti) ? sim_t : -inf   (rows with no causal keep sim_t)
  top-7 by masked value -> cross-attn fusion -> deviation score  [Bq]

Strategy (8 NeuronCores, pool-sharded):
  * For causal pairs exp(-l*|ti-tj|) = exp(-l*ti)*exp(l*tj): fold the decay
    and the L2 norms into the matmul operands on the host (non-causal pairs
    get a wrong decay but are masked out on the host anyway).
  * Operands are quantized to fp8e4m3 (scale 64) on the host; the device
    runs DoubleRow fp8 matmuls (K=256 in one instruction at 2x bf16 rate).
  * Sort pool by time, shard round-robin across 8 cores (8192 items/core,
    balanced); sort queries by time. The causal mask becomes a column-prefix
    per row; only the causal prefix (rounded to 128 cols) of each 128-query
    tile is computed.
  * Device per core: fp8 DoubleRow matmuls -> PSUM fp32; window-max of 8
    time-contiguous columns -> fp16 band. The reduction alternates between
    two paths to balance engines: ACT stages the chunk to SBUF fp16 and DVE
    does an 8:1 tensor_reduce (2-byte packed = fast DVE mode), or DVE
    reduces directly from PSUM fp32. Band [2048, <=1024] fp16 DMA'd out.
  * Host: applies the exact causal window kill, takes each row's global
    top-(7+margin) window-max threshold with an fp8-error slack, rescores
    the selected windows' columns exactly in float64, selects top-7 with
    reference tie semantics, and computes the softmax fusion + anomaly
    score (trivial FLOPs).
"""

import numpy as np

BQ, BN, H, K = 2048, 65536, 256, 7
NCORES = 8
LAMBDA = 0.1
GAMMA, DELTA = 0.5, 0.5
EPS = 1e-12
COS_EPS = 1e-8
CHUNK = 512
PTGRAN = 64  # causal-prefix granularity (cols)
SHARD = BN // NCORES  # 8192
QTILE = 128
NTILES = BQ // QTILE  # 16
WIN = 8
NWIN_MAX = SHARD // WIN  # 1024
FP8_SCALE = 64.0
MARGIN = 12  # extra windows beyond K in the host threshold selection
# absolute slack on the window threshold, in unscaled cos units, covering
# fp8 operand quantization error of the 256-term dot products
SLACK_ABS = 1.2e-2
MAXW_ROW = 64  # cap on host-selected windows per row before full fallback
# chunk reduction path pattern: True = ACT-stage + DVE fp16 reduce,
# False = DVE direct PSUM reduce. PSUM reads serialize across engines on
# TRN2 (~1.16 ns/elem shared port), so a single PSUM reader (DVE direct)
# measured fastest: mixing in ACT stages only adds per-instruction overhead.
RED_PATTERN = (False,)

_PROGRAM_CACHE = {}


def _build_program(pt_list, reps=1, hw_loop=0):
    import contextlib

    import concourse.bacc as bacc
    import concourse.mybir as mybir
    import concourse.tile as tile

    f32 = mybir.dt.float32
    f16 = mybir.dt.float16
    fp8 = mybir.dt.float8e4
    DR = mybir.MatmulPerfMode.DoubleRow
    MAXOP = mybir.AluOpType.max
    AXX = mybir.AxisListType.X

    nc = bacc.Bacc("TRN2", target_bir_lowering=False, debug=False)

    q_d = nc.dram_tensor("qT", [128, 2, BQ], fp8, kind="ExternalInput")
    p_d = nc.dram_tensor("pT", [128, 2, SHARD], fp8, kind="ExternalInput")
    wb_d = nc.dram_tensor("wb", [BQ, NWIN_MAX], f16, kind="ExternalOutput")

    with tile.TileContext(nc) as tc:
        with (
            tc.tile_pool(name="resident", bufs=1) as resp,
            tc.tile_pool(name="wband", bufs=3) as wbandp,
            tc.tile_pool(name="stage", bufs=4) as stagep,
            tc.tile_pool(name="psum", bufs=4, space="PSUM") as psump,
        ):
          with tc.For_i(0, hw_loop, 1) if hw_loop else contextlib.nullcontext():
            for _rep in range(reps):
              q_sb = resp.tile([128, 2, BQ], fp8, tag="q", name="q")
              p_sb = resp.tile([128, 2, SHARD], fp8, tag="p", name="p")
              # queries via sync-engine DMA, tile 1's stationary slice
              # first (tile 1 is processed first); pool pieces via the scalar
              # engine's hardware DGE (ACT is otherwise idle and HWDGE setup
              # is ~3x faster than gpsimd SWDGE); small leading pieces land
              # the first chunks sooner so matmuls start early
              nc.sync.dma_start(
                  q_sb[:, :, QTILE : 2 * QTILE], q_d[:, :, QTILE : 2 * QTILE]
              )
              nc.sync.dma_start(q_sb[:, :, :QTILE], q_d[:, :, :QTILE])
              nc.sync.dma_start(
                  q_sb[:, :, 2 * QTILE :], q_d[:, :, 2 * QTILE :]
              )
              p_pieces = [512, 512] + [1024] * 7
              c0 = 0
              for plen in p_pieces:
                  nc.scalar.dma_start(
                      p_sb[:, :, c0 : c0 + plen], p_d[:, :, c0 : c0 + plen]
                  )
                  c0 += plen

              # process the smallest tile last to shrink the kernel-tail drain
              tile_order = list(range(1, NTILES)) + [0]
              GW = 2 * CHUNK  # 2 chunks per PSUM tile (2 banks); 1 reduce each
              for t in tile_order:
                  pt_len = pt_list[t]
                  nwin = pt_len // WIN
                  wband = wbandp.tile([QTILE, NWIN_MAX], f16, tag="wband")
                  for g0 in range(0, pt_len, GW):
                      gw = min(GW, pt_len - g0)
                      ps = psump.tile([QTILE, GW], f32, tag="ps")
                      for c0 in range(0, gw, CHUNK):
                          w = min(CHUNK, gw - c0)
                          nc.tensor.matmul(
                              ps[:, c0 : c0 + w],
                              q_sb[:, :, t * QTILE : (t + 1) * QTILE],
                              p_sb[:, :, g0 + c0 : g0 + c0 + w],
                              start=True,
                              stop=True,
                              perf_mode=DR,
                              skip_group_check=True,
                          )
                      # one DVE window-max per group, straight from PSUM: PSUM
                      # reads serialize across engines (~1.16 ns/elem shared
                      # port), so DVE is the sole PSUM reader and bigger
                      # reduces amortize the per-instruction port overhead
                      nc.vector.tensor_reduce(
                          out=wband[:, g0 // WIN : (g0 + gw) // WIN],
                          in_=ps[:, :gw].rearrange("p (b x) -> p b x", x=WIN),
                          axis=AXX,
                          op=MAXOP,
                      )

                  nc.sync.dma_start(
                      wb_d[t * QTILE : (t + 1) * QTILE, :nwin], wband[:, :nwin]
                  )

    nc.compile()
    return nc


def _prepare(query_emb, query_time, pool_emb, pool_time):
    """Host preprocessing: fold norms+decay into operands, sort, shard,
    quantize to fp8e4m3, lay out as [128, 2, N] (DoubleRow k-subtiles)."""
    import ml_dtypes

    q = query_emb.astype(np.float64)
    p = pool_emb.astype(np.float64)
    qt = query_time.astype(np.float64)
    pt = pool_time.astype(np.float64)

    qnorm = np.linalg.norm(q, axis=1)
    pnorm = np.linalg.norm(p, axis=1)
    qs = (q / np.maximum(qnorm, EPS)[:, None]) * np.exp(-LAMBDA * qt)[:, None]
    ps = (p / np.maximum(pnorm, EPS)[:, None]) * np.exp(LAMBDA * pt)[:, None]

    pperm = np.argsort(pool_time, kind="stable")
    qperm = np.argsort(query_time, kind="stable")
    ps_sorted = ps[pperm]
    pt_sorted = pool_time[pperm]
    qs_sorted = qs[qperm]
    qt_sorted = query_time[qperm]

    fp8t = ml_dtypes.float8_e4m3
    q8 = (qs_sorted * FP8_SCALE).astype(np.float32).astype(fp8t)  # [BQ, 256]
    # [128, 2, BQ]: element (p, i, b) = q8[b, i*128+p]
    qT8 = np.ascontiguousarray(q8.T.reshape(2, 128, BQ).transpose(1, 0, 2))

    shard_emb = []
    shard_times = []
    for k in range(NCORES):
        s8 = (ps_sorted[k::NCORES] * FP8_SCALE).astype(np.float32).astype(fp8t)
        shard_emb.append(
            np.ascontiguousarray(s8.T.reshape(2, 128, SHARD).transpose(1, 0, 2))
        )
        shard_times.append(pt_sorted[k::NCORES])
    # exact count of shard items with tj < ti (strict), per core per sorted query
    ci = np.stack(
        [np.searchsorted(shard_times[k], qt_sorted, side="left") for k in range(NCORES)]
    ).astype(np.int64)  # [8, 2048]

    return qT8, shard_emb, ci, pperm, qperm


def _pt_list(ci):
    ci_tiles = ci.reshape(NCORES, NTILES, QTILE)
    maxci = ci_tiles.max(axis=0).max(axis=1)  # [NTILES]
    return np.clip(
        np.ceil(maxci / PTGRAN).astype(np.int64) * PTGRAN, PTGRAN, SHARD
    ).tolist()


def _core_in_map(qT8, shard_emb, k):
    return {"qT": qT8, "pT": shard_emb[k]}


def _emulate_windows(qT8, shard_emb, pt_list):
    """Numerically emulate the device program on host (for calibration)."""
    wb = np.zeros((NCORES, BQ, NWIN_MAX), dtype=np.float16)
    nwin_t = [pt_list[t] // WIN for t in range(NTILES)]
    q32 = qT8.astype(np.float32).transpose(1, 0, 2).reshape(256, BQ)  # [256, BQ]
    for k in range(NCORES):
        p32 = shard_emb[k].astype(np.float32).transpose(1, 0, 2).reshape(256, SHARD)
        for t in range(NTILES):
            pl = pt_list[t]
            sims = (
                q32[:, t * QTILE : (t + 1) * QTILE].T @ p32[:, :pl]
            )  # [128, pl] f32
            wmax = sims.reshape(QTILE, pl // WIN, WIN).max(axis=2)
            wb[k, t * QTILE : (t + 1) * QTILE, : nwin_t[t]] = wmax.astype(np.float16)
    return wb


def _device_windows(qT8, shard_emb, ci, emulate=False):
    """Run the Bass kernel; return per-core window-max bands [8, 2048, 1024]."""
    pt_list = _pt_list(ci)
    if emulate:
        return _emulate_windows(qT8, shard_emb, pt_list), pt_list

    from concourse.bass_utils import run_bass_kernel_spmd

    key = tuple(pt_list)
    if key not in _PROGRAM_CACHE:
        _PROGRAM_CACHE.clear()
        _PROGRAM_CACHE[key] = _build_program(pt_list)
    nc = _PROGRAM_CACHE[key]

    in_maps = [_core_in_map(qT8, shard_emb, k) for k in range(NCORES)]
    res = run_bass_kernel_spmd(nc, in_maps, core_ids=list(range(NCORES)))
    wb = np.stack([res.results[k]["wb"] for k in range(NCORES)])  # [8, 2048, 1024]
    return wb, pt_list


def _merge_and_score(
    wb, pt_list, ci, pperm, qperm, query_emb, query_time, pool_emb, pool_time
):
    """Select candidate windows by global threshold, rescore exactly, score."""
    nq = BQ
    wmin = WIN * np.arange(NWIN_MAX, dtype=np.int64)  # window min time-col

    # validity: window exists for the row's tile and contains >=1 causal col
    nwin_row = (np.asarray(pt_list, dtype=np.int64) // WIN)[
        np.repeat(np.arange(NTILES), QTILE)
    ]  # [2048]
    exists = np.arange(NWIN_MAX)[None, :] < nwin_row[:, None]  # [2048, 1024]
    wbf = wb.astype(np.float32)
    wbm = np.where(
        exists[None, :, :] & (wmin[None, None, :] < ci[:, :, None]),
        wbf,
        -np.inf,
    )  # [8, 2048, 1024]

    flat = np.transpose(wbm, (1, 0, 2)).reshape(nq, NCORES * NWIN_MAX)
    KM = K + MARGIN
    kth = np.partition(flat, -KM, axis=1)[:, -KM]  # (K+MARGIN)-th largest
    # slack: fp8 quantization error of the dot products (scaled units) plus
    # a relative term for fp16 band storage rounding
    slack = SLACK_ABS * FP8_SCALE * FP8_SCALE + np.abs(kth) * 2.0**-10 + 1e-6
    kth = kth - slack
    # rows with fewer than K+MARGIN valid windows: select all valid ones
    thr = np.where(np.isfinite(kth), kth, -1.0e38)
    sel = flat >= thr[:, None]
    nsel = sel.sum(axis=1)

    rows, wcols = np.nonzero(sel)
    core = wcols // NWIN_MAX
    w = wcols % NWIN_MAX
    # candidate columns: global time-sorted position -> original pool index
    cols_shard = (WIN * w)[:, None] + np.arange(WIN)[None, :]  # [nsel, WIN]
    sorted_pos = cols_shard * NCORES + core[:, None]
    orig = pperm[sorted_pos]  # [nsel_total, WIN] original pool rows

    # exact rescore in float64
    q64 = query_emb.astype(np.float64)
    qn64 = q64 / np.maximum(np.linalg.norm(q64, axis=1), EPS)[:, None]
    pnorm = np.linalg.norm(pool_emb.astype(np.float64), axis=1)
    oi_rows = qperm[rows]  # original query row per selected window
    n_ent = rows.shape[0]
    sims = np.empty((n_ent, WIN), dtype=np.float64)
    causal = np.empty((n_ent, WIN), dtype=bool)
    BLK = 65536
    for b in range(0, n_ent, BLK):
        sl = slice(b, b + BLK)
        emb = pool_emb[orig[sl]].astype(np.float64)  # [blk, WIN, 256]
        pn = np.maximum(pnorm[orig[sl]], EPS)
        dots = np.einsum("nh,nch->nc", qn64[oi_rows[sl]], emb) / pn
        tdiff = np.abs(
            query_time[oi_rows[sl]].astype(np.float64)[:, None]
            - pool_time[orig[sl]].astype(np.float64)
        )
        sims[sl] = dots * np.exp(-LAMBDA * tdiff)
        causal[sl] = pool_time[orig[sl]] < query_time[oi_rows[sl]][:, None]

    # scatter into dense per-row candidate arrays
    maxw = max(min(int(nsel.max()), MAXW_ROW), 1)
    slot = np.zeros(n_ent, dtype=np.int64)
    if n_ent:
        # rows is sorted; position of each entry within its row
        row_start = np.searchsorted(rows, np.arange(nq), side="left")
        slot = np.arange(n_ent) - row_start[rows]
    keep = slot < MAXW_ROW
    dsims = np.full((nq, maxw * WIN), -np.inf)
    dorig = np.zeros((nq, maxw * WIN), dtype=np.int64)
    rk = rows[keep]
    sk = slot[keep]
    for o in range(WIN):
        dsims[rk, sk * WIN + o] = np.where(causal[keep, o], sims[keep, o], -np.inf)
        dorig[rk, sk * WIN + o] = orig[keep, o]

    # order by fp32-rounded sims to match the reference's float32 top_k tie
    # semantics (ties break to the lowest original pool index)
    ds32 = dsims.astype(np.float32).astype(np.float64)
    order2 = np.lexsort((dorig, -ds32), axis=1)[:, :K]
    top_idx = np.take_along_axis(dorig, order2, axis=1)
    nvalid_row = np.isfinite(np.take_along_axis(dsims, order2, axis=1)).sum(axis=1)

    # rows needing the exact slow path
    pt_min = pool_time.min()
    n_causal_global = np.searchsorted(
        np.sort(pool_time), query_time[qperm], side="left"
    )
    fix_rows = np.nonzero(
        (query_time[qperm] <= pt_min)
        | (np.minimum(n_causal_global, K) > nvalid_row)
        | (n_causal_global < K)
        | (nsel > MAXW_ROW)
    )[0]
    if len(fix_rows):
        pn_all = pool_emb.astype(np.float64) / np.maximum(pnorm, EPS)[:, None]
    for i in fix_rows:
        oi = qperm[i]
        ti = query_time[oi]
        sims_all = (pn_all @ qn64[oi]) * np.exp(
            -LAMBDA * np.abs(float(ti) - pool_time.astype(np.float64))
        )
        if ti <= pt_min:
            # row_all_inf: reference keeps unmasked decayed sims
            top_idx[i] = np.argsort(-sims_all.astype(np.float32), kind="stable")[:K]
            continue
        causal_all = pool_time < ti
        c = int(causal_all.sum())
        masked_all = np.where(causal_all, sims_all, -np.inf).astype(np.float32)
        picks = list(np.argsort(-masked_all, kind="stable")[: min(c, K)])
        # pad like jax.lax.top_k over -inf ties: lowest non-causal original idx
        j = 0
        while len(picks) < K:
            if not causal_all[j]:
                picks.append(j)
            j += 1
        top_idx[i] = np.array(picks, dtype=np.int64)

    # fusion + score in float64 (reference is f32; fp64 is strictly closer)
    q = query_emb.astype(np.float64)[qperm]  # sorted-query order
    retrieved = pool_emb.astype(np.float64)[top_idx]  # [2048, 7, 256]
    scale = float(H) ** -0.5
    logits = np.einsum("bh,bkh->bk", q, retrieved) * scale
    logits -= logits.max(axis=1, keepdims=True)
    e = np.exp(logits)
    attn = e / e.sum(axis=1, keepdims=True)
    fused = np.einsum("bk,bkh->bh", attn, retrieved)

    qn2 = np.linalg.norm(q, axis=1)
    fn2 = np.linalg.norm(fused, axis=1)
    cos = np.sum(q * fused, axis=1) / np.maximum(qn2 * fn2, COS_EPS)
    l2 = np.linalg.norm(q - fused, axis=1)
    score_sorted = GAMMA * (1.0 - cos) + DELTA * l2

    out = np.zeros(nq, dtype=np.float32)
    out[qperm] = score_sorted.astype(np.float32)
    return out


def kernel(query_emb, query_time, pool_emb, pool_time, _emulate=False):
    query_emb = np.asarray(query_emb, dtype=np.float32)
    query_time = np.asarray(query_time, dtype=np.float32)
    pool_emb = np.asarray(pool_emb, dtype=np.float32)
    pool_time = np.asarray(pool_time, dtype=np.float32)

    qT8, shard_emb, ci, pperm, qperm = _prepare(
        query_emb, query_time, pool_emb, pool_time
    )
    wb, pt_list = _device_windows(qT8, shard_emb, ci, emulate=_emulate)
    return _merge_and_score(
        wb, pt_list, ci, pperm, qperm, query_emb, query_time, pool_emb, pool_time
    )

